# revision 17
# baseline (speedup 1.0000x reference)
"""Trainium2 Bass kernel for AttentionPoolCompressor.

Computation (matches the reference nn.Module):
    x = chunk.reshape(N, 4, 512)
    scores = einsum('d,nrd->nr', query, x) / sqrt(512)
    attn   = softmax(scores, axis=-1)
    pooled = einsum('nr,nrd->nd', attn, x)
    out    = pooled @ w.T + b

Sharding: chunk rows are split contiguously across 8 NeuronCores (each pools
its own L/8 rows independently); query / w / b are replicated.  No
collectives; each core writes its own slice of the output.

v3 design (memory-regime target ~14us per 2048-row tile):
  * 2048-row tiles (16/core).  Input DMA is SWDGE (gpsimd) with an inline
    fp32->bf16 cast: partition p holds 16 consecutive rows = one contiguous
    32KB HBM descriptor.  Output DMA is HWDGE (sync), 8KB/partition.
  * Stage->engine split chosen from measured rates (DVE bf16 TT 0.65ns/e,
    DVE reduce 1.29, ACT 1.6, GpSimd 2.7):
      DVE: q-product (one bf16 TT, 2x mode), pair-sum tree (two bf16 TT
           adds) + short fp32 tensor_reduce, softmax small ops.
      ACT: exp, pooled/pooledT/out PSUM<->SBUF moves.
      GpSimd: input-DMA descriptor emission, attn-diag build.
      PE:  pool (diag matmuls), transposes, projection - all bf16 1c/row -
           and the bias as a K=1 ones x b matmul in the proj PSUM group.
  * Software pipelining: iteration i runs score stages for tile i but the
    PE/copy block for tile i-1 and exp(i) dead last, so every engine queue
    only ever waits on work that is already done (v1/v2 lost 2-3x to
    queue-head stalls).
"""

import math
import sys

import numpy as np

if "/opt/trn_rl_repo" not in sys.path:
    sys.path.insert(0, "/opt/trn_rl_repo")

D = 512
RATIO = 4
N_CORES = 8
L_FULL = 262144
ROWS_PER_CORE = L_FULL // N_CORES  # 32768
TILE_ROWS = 2048  # input rows per tile -> 512 output rows
J = TILE_ROWS // 128  # row-groups per partition (16)
G = J // RATIO  # softmax groups per partition (4)

_NC_CACHE = {}


def _build_nc(rows_per_core, reps=1):
    import contextlib
    from contextlib import ExitStack

    import concourse.bacc as bacc
    import concourse.tile as tile
    from concourse import mybir

    fp32 = mybir.dt.float32
    bf16 = mybir.dt.bfloat16
    Alu = mybir.AluOpType
    Act = mybir.ActivationFunctionType
    X = mybir.AxisListType.X

    n_tiles = rows_per_core // TILE_ROWS
    out_rows = rows_per_core // RATIO
    inv_sqrt_d = 1.0 / math.sqrt(D)

    nc = bacc.Bacc("TRN2", target_bir_lowering=False, debug=False)
    chunk = nc.dram_tensor("chunk", [rows_per_core, D], fp32, kind="ExternalInput").ap()
    wtb = nc.dram_tensor("wtb", [D, D], bf16, kind="ExternalInput").ap()
    qbc = nc.dram_tensor("qbc", [128, D], bf16, kind="ExternalInput").ap()
    ident = nc.dram_tensor("ident", [128, 128], bf16, kind="ExternalInput").ap()
    identf = nc.dram_tensor("identf", [128, 128], fp32, kind="ExternalInput").ap()
    ones1 = nc.dram_tensor("ones1", [1, 128], bf16, kind="ExternalInput").ap()
    brow = nc.dram_tensor("brow", [1, D], bf16, kind="ExternalInput").ap()
    out = nc.dram_tensor("out", [out_rows, D], fp32, kind="ExternalOutput").ap()

    with tile.TileContext(nc) as tc, ExitStack() as ctx:
        const = ctx.enter_context(tc.tile_pool(name="const", bufs=1))
        xp = ctx.enter_context(tc.tile_pool(name="xp", bufs=5))
        pp = ctx.enter_context(tc.tile_pool(name="pp", bufs=1))
        t1p = ctx.enter_context(tc.tile_pool(name="t1p", bufs=1))
        s_p = ctx.enter_context(tc.tile_pool(name="s_p", bufs=3))
        e_p = ctx.enter_context(tc.tile_pool(name="e_p", bufs=3))
        gs_p = ctx.enter_context(tc.tile_pool(name="gs_p", bufs=3))
        rec_p = ctx.enter_context(tc.tile_pool(name="rec_p", bufs=3))
        dp = ctx.enter_context(tc.tile_pool(name="dp", bufs=3))
        pooledp = ctx.enter_context(tc.tile_pool(name="pooledp", bufs=2))
        ptp = ctx.enter_context(tc.tile_pool(name="ptp", bufs=2))
        outp = ctx.enter_context(tc.tile_pool(name="outp", bufs=3))
        ps_pool = ctx.enter_context(tc.tile_pool(name="ps_pool", bufs=2, space="PSUM"))
        ps_pt = ctx.enter_context(tc.tile_pool(name="ps_pt", bufs=2, space="PSUM"))
        ps_o = ctx.enter_context(tc.tile_pool(name="ps_o", bufs=4, space="PSUM"))

        # Constants (replicated small tensors)
        wt_t = const.tile([128, 4 * D], bf16)
        for c in range(4):
            nc.sync.dma_start(
                out=wt_t[:, c * D : (c + 1) * D], in_=wtb[c * 128 : (c + 1) * 128, :]
            )
        q_t = const.tile([128, D], bf16)
        nc.sync.dma_start(out=q_t[:], in_=qbc[:, :])
        id_t = const.tile([128, 128], bf16)
        nc.sync.dma_start(out=id_t[:], in_=ident[:, :])
        idf_t = const.tile([128, 128], fp32)
        nc.sync.dma_start(out=idf_t[:], in_=identf[:, :])
        ones_t = const.tile([1, 128], bf16)
        nc.sync.dma_start(out=ones_t[:], in_=ones1[:, :])
        b_t = const.tile([1, D], bf16)
        nc.sync.dma_start(out=b_t[:], in_=brow[:, :])

        def load_tile(t):
            x_t = xp.tile([128, J * D], bf16)
            nc.gpsimd.dma_start(
                out=x_t[:],
                in_=chunk[t * TILE_ROWS : (t + 1) * TILE_ROWS, :].rearrange(
                    "(p j) d -> p (j d)", j=J
                ),
            )
            return x_t

        def scores_front(t, st):
            """DVE: q-product + pair-sum tree + short reduce -> s_t [128,J]."""
            x_t = st["x"]
            prod = pp.tile([128, J * D], bf16)
            nc.vector.tensor_tensor(
                prod[:].rearrange("p (j d) -> p j d", j=J),
                x_t[:].rearrange("p (j d) -> p j d", j=J),
                q_t[:].unsqueeze(1).broadcast_to((128, J, D)),
                Alu.mult,
            )
            h1 = D // 2
            tr1 = t1p.tile([128, J * h1], bf16)
            v1 = tr1[:].rearrange("p (j d) -> p j d", j=J)
            pv = prod[:].rearrange("p (j d) -> p j d", j=J)
            nc.vector.tensor_tensor(v1, pv[:, :, 0:h1], pv[:, :, h1:D], Alu.add)
            s_t = s_p.tile([128, J], fp32)
            nc.vector.tensor_reduce(s_t[:], v1, axis=X, op=Alu.add)
            st["s"] = s_t

        def exp_stage(t, st):
            """ACT exp (unnormalized attn weights, bf16)."""
            e_t = e_p.tile([128, J], fp32)
            nc.scalar.activation(
                out=e_t[:], in_=st["s"], func=Act.Exp, scale=inv_sqrt_d
            )
            st["e"] = e_t

        def softmax_finish(t, st):
            """DVE group sums + reciprocal (normalization happens via the
            per-partition scale on the pooled PSUM->SBUF copy); GpSimd builds
            the UNNORMALIZED diag from exp values."""
            e_t = st["e"]
            gs_t = gs_p.tile([128, G], fp32)
            nc.vector.tensor_reduce(
                gs_t[:], e_t[:].rearrange("p (g r) -> p g r", g=G), axis=X, op=Alu.add
            )
            rec_t = rec_p.tile([128, G], fp32)
            nc.vector.reciprocal(rec_t[:], gs_t[:])
            st["rec"] = rec_t

        DVE_DJ = 8  # diag row-groups built on DVE; the rest on ACT

        def d_stage_dve(t, st):
            d_t = dp.tile([128, J * 128], bf16)
            nc.vector.tensor_tensor(
                d_t[:, : DVE_DJ * 128].rearrange("p (j m) -> p j m", j=DVE_DJ),
                idf_t[:].unsqueeze(1).broadcast_to((128, DVE_DJ, 128)),
                st["e"][:, :DVE_DJ].unsqueeze(2).broadcast_to((128, DVE_DJ, 128)),
                Alu.mult,
            )
            st["d"] = d_t

        def d_stage_act(t, st):
            d_t = st["d"]
            for j in range(DVE_DJ, J):
                nc.scalar.activation(
                    out=d_t[:, j * 128 : (j + 1) * 128],
                    in_=id_t[:],
                    func=Act.Copy,
                    scale=st["e"][:, j : j + 1],
                )

        def pe_block(t, st):
            """PE pool/transpose/proj(+bias) with ACT moves; out store."""
            x_t, d_t, rec_t = st["x"], st["d"], st["rec"]
            out_sb = outp.tile([128, G * D], fp32)
            for g in range(G):
                pool_ps = ps_pool.tile([128, D], fp32)
                for r in range(RATIO):
                    j = g * RATIO + r
                    nc.tensor.matmul(
                        out=pool_ps[:],
                        lhsT=d_t[:, j * 128 : (j + 1) * 128],
                        rhs=x_t[:, j * D : (j + 1) * D],
                        start=(r == 0),
                        stop=(r == RATIO - 1),
                    )
                pooled_sb = pooledp.tile([128, D], bf16)
                nc.scalar.activation(
                    out=pooled_sb[:],
                    in_=pool_ps[:],
                    func=Act.Copy,
                    scale=rec_t[:, g : g + 1],
                )

                pt_ps = ps_pt.tile([128, D], bf16)
                for c in range(4):
                    nc.tensor.transpose(
                        pt_ps[:, c * 128 : (c + 1) * 128],
                        pooled_sb[:, c * 128 : (c + 1) * 128],
                        id_t[:],
                    )
                pt_sb = ptp.tile([128, D], bf16)
                nc.scalar.copy(pt_sb[:], pt_ps[:])

                o_ps = ps_o.tile([128, D], fp32)
                nc.tensor.matmul(
                    out=o_ps[:], lhsT=ones_t[:], rhs=b_t[:], start=True, stop=False
                )
                for c in range(4):
                    nc.tensor.matmul(
                        out=o_ps[:],
                        lhsT=pt_sb[:, c * 128 : (c + 1) * 128],
                        rhs=wt_t[:, c * D : (c + 1) * D],
                        start=False,
                        stop=(c == 3),
                    )
                nc.scalar.copy(out_sb[:, g * D : (g + 1) * D], o_ps[:])
            nc.sync.dma_start(
                out=out[t * 512 : (t + 1) * 512, :].rearrange(
                    "(p j) d -> p (j d)", j=G
                ),
                in_=out_sb[:],
            )

        rep_loop = tc.For_i(0, reps, 1) if reps > 1 else contextlib.nullcontext()
        with rep_loop:
            states = {}
            PREFETCH = 2
            for t in range(min(PREFETCH, n_tiles)):
                states[t] = {"x": load_tile(t)}
            for i in range(n_tiles + 2):
                if i + PREFETCH < n_tiles:
                    states[i + PREFETCH] = {"x": load_tile(i + PREFETCH)}
                if 1 <= i <= n_tiles:
                    # Tile i-1 softmax stages: deps one iteration old, so
                    # exp fires the moment ACT reaches it.
                    exp_stage(i - 1, states[i - 1])
                    softmax_finish(i - 1, states[i - 1])
                    d_stage_dve(i - 1, states[i - 1])
                    d_stage_act(i - 1, states[i - 1])
                if i < n_tiles:
                    scores_front(i, states[i])
                if i >= 2:
                    # PE/copy block lags TWO tiles so its ACT copies never
                    # block the next exp at the ACT queue head.
                    pe_block(i - 2, states[i - 2])
                    del states[i - 2]

    nc.compile()
    return nc


def get_nc(rows_per_core=ROWS_PER_CORE, reps=1):
    key = (rows_per_core, reps)
    if key not in _NC_CACHE:
        _NC_CACHE[key] = _build_nc(rows_per_core, reps)
    return _NC_CACHE[key]


def _aux_inputs(query, w, b):
    import ml_dtypes

    bf16 = ml_dtypes.bfloat16
    q = np.asarray(query, dtype=np.float32)
    qbc = np.ascontiguousarray(np.broadcast_to(q.astype(bf16), (128, D)))
    wtb = np.ascontiguousarray(np.asarray(w, dtype=np.float32).T.astype(bf16))
    ident = np.eye(128, dtype=bf16)
    identf = np.eye(128, dtype=np.float32)
    ones1 = np.ones((1, 128), dtype=bf16)
    brow = np.asarray(b, dtype=np.float32).astype(bf16).reshape(1, D)
    return {
        "qbc": qbc,
        "wtb": wtb,
        "ident": ident,
        "identf": identf,
        "ones1": ones1,
        "brow": brow,
    }


def make_in_maps(chunk, query, w, b, rows_per_core=ROWS_PER_CORE, n_cores=N_CORES):
    chunk = np.asarray(chunk, dtype=np.float32)
    aux = _aux_inputs(query, w, b)
    return [
        {
            "chunk": np.ascontiguousarray(
                chunk[c * rows_per_core : (c + 1) * rows_per_core]
            ),
            **aux,
        }
        for c in range(n_cores)
    ]


def kernel(chunk, query, w, b, trace=False):
    from concourse.bass_utils import run_bass_kernel_spmd

    nc = get_nc(ROWS_PER_CORE)
    in_maps = make_in_maps(chunk, query, w, b)
    res = run_bass_kernel_spmd(nc, in_maps, list(range(N_CORES)), trace=trace)
    out = np.concatenate([res.results[c]["out"] for c in range(N_CORES)], axis=0)
    kernel.last_results = res
    return out


# revision 18
# speedup vs baseline: 1.0292x; 1.0292x over previous
"""Trainium2 Bass kernel for AttentionPoolCompressor.

Computation (matches the reference nn.Module):
    x = chunk.reshape(N, 4, 512)
    scores = einsum('d,nrd->nr', query, x) / sqrt(512)
    attn   = softmax(scores, axis=-1)
    pooled = einsum('nr,nrd->nd', attn, x)
    out    = pooled @ w.T + b

Sharding: chunk rows are split contiguously across 8 NeuronCores (each pools
its own L/8 rows independently); query / w / b are replicated.  No
collectives; each core writes its own slice of the output.

v3 design (memory-regime target ~14us per 2048-row tile):
  * 2048-row tiles (16/core).  Input DMA is SWDGE (gpsimd) with an inline
    fp32->bf16 cast: partition p holds 16 consecutive rows = one contiguous
    32KB HBM descriptor.  Output DMA is HWDGE (sync), 8KB/partition.
  * Stage->engine split chosen from measured rates (DVE bf16 TT 0.65ns/e,
    DVE reduce 1.29, ACT 1.6, GpSimd 2.7):
      DVE: q-product (one bf16 TT, 2x mode), pair-sum tree (two bf16 TT
           adds) + short fp32 tensor_reduce, softmax small ops.
      ACT: exp, pooled/pooledT/out PSUM<->SBUF moves.
      GpSimd: input-DMA descriptor emission, attn-diag build.
      PE:  pool (diag matmuls), transposes, projection - all bf16 1c/row -
           and the bias as a K=1 ones x b matmul in the proj PSUM group.
  * Software pipelining: iteration i runs score stages for tile i but the
    PE/copy block for tile i-1 and exp(i) dead last, so every engine queue
    only ever waits on work that is already done (v1/v2 lost 2-3x to
    queue-head stalls).
"""

import math
import sys

import numpy as np

if "/opt/trn_rl_repo" not in sys.path:
    sys.path.insert(0, "/opt/trn_rl_repo")

D = 512
RATIO = 4
N_CORES = 8
L_FULL = 262144
ROWS_PER_CORE = L_FULL // N_CORES  # 32768
TILE_ROWS = 2048  # input rows per tile -> 512 output rows
J = TILE_ROWS // 128  # row-groups per partition (16)
G = J // RATIO  # softmax groups per partition (4)

_NC_CACHE = {}


def _build_nc(rows_per_core, reps=1):
    import contextlib
    from contextlib import ExitStack

    import concourse.bacc as bacc
    import concourse.tile as tile
    from concourse import mybir

    fp32 = mybir.dt.float32
    bf16 = mybir.dt.bfloat16
    Alu = mybir.AluOpType
    Act = mybir.ActivationFunctionType
    X = mybir.AxisListType.X

    n_tiles = rows_per_core // TILE_ROWS
    out_rows = rows_per_core // RATIO
    inv_sqrt_d = 1.0 / math.sqrt(D)

    nc = bacc.Bacc("TRN2", target_bir_lowering=False, debug=False)
    chunk = nc.dram_tensor("chunk", [rows_per_core, D], fp32, kind="ExternalInput").ap()
    wtb = nc.dram_tensor("wtb", [D, D], bf16, kind="ExternalInput").ap()
    qbc = nc.dram_tensor("qbc", [128, D], bf16, kind="ExternalInput").ap()
    ident = nc.dram_tensor("ident", [128, 128], bf16, kind="ExternalInput").ap()
    identf = nc.dram_tensor("identf", [128, 128], fp32, kind="ExternalInput").ap()
    ones1 = nc.dram_tensor("ones1", [1, 128], bf16, kind="ExternalInput").ap()
    brow = nc.dram_tensor("brow", [1, D], bf16, kind="ExternalInput").ap()
    out = nc.dram_tensor("out", [out_rows, D], fp32, kind="ExternalOutput").ap()

    with tile.TileContext(nc) as tc, ExitStack() as ctx:
        const = ctx.enter_context(tc.tile_pool(name="const", bufs=1))
        xp = ctx.enter_context(tc.tile_pool(name="xp", bufs=5))
        pp = ctx.enter_context(tc.tile_pool(name="pp", bufs=1))
        t1p = ctx.enter_context(tc.tile_pool(name="t1p", bufs=1))
        s_p = ctx.enter_context(tc.tile_pool(name="s_p", bufs=3))
        e_p = ctx.enter_context(tc.tile_pool(name="e_p", bufs=3))
        gs_p = ctx.enter_context(tc.tile_pool(name="gs_p", bufs=3))
        rec_p = ctx.enter_context(tc.tile_pool(name="rec_p", bufs=3))
        dp = ctx.enter_context(tc.tile_pool(name="dp", bufs=3))
        pooledp = ctx.enter_context(tc.tile_pool(name="pooledp", bufs=2))
        ptp = ctx.enter_context(tc.tile_pool(name="ptp", bufs=2))
        outp = ctx.enter_context(tc.tile_pool(name="outp", bufs=3))
        stagep = ctx.enter_context(tc.tile_pool(name="stagep", bufs=2))
        ps_pool = ctx.enter_context(tc.tile_pool(name="ps_pool", bufs=2, space="PSUM"))
        ps_pt = ctx.enter_context(tc.tile_pool(name="ps_pt", bufs=2, space="PSUM"))
        ps_o = ctx.enter_context(tc.tile_pool(name="ps_o", bufs=4, space="PSUM"))

        # Constants (replicated small tensors)
        wt_t = const.tile([128, 4 * D], bf16)
        for c in range(4):
            nc.sync.dma_start(
                out=wt_t[:, c * D : (c + 1) * D], in_=wtb[c * 128 : (c + 1) * 128, :]
            )
        q_t = const.tile([128, D], bf16)
        nc.sync.dma_start(out=q_t[:], in_=qbc[:, :])
        id_t = const.tile([128, 128], bf16)
        nc.sync.dma_start(out=id_t[:], in_=ident[:, :])
        idf_t = const.tile([128, 128], fp32)
        nc.sync.dma_start(out=idf_t[:], in_=identf[:, :])
        ones_t = const.tile([1, 128], bf16)
        nc.sync.dma_start(out=ones_t[:], in_=ones1[:, :])
        b_t = const.tile([1, D], bf16)
        nc.sync.dma_start(out=b_t[:], in_=brow[:, :])

        def load_tile(t):
            x_t = xp.tile([128, J * D], bf16)
            nc.gpsimd.dma_start(
                out=x_t[:],
                in_=chunk[t * TILE_ROWS : (t + 1) * TILE_ROWS, :].rearrange(
                    "(p j) d -> p (j d)", j=J
                ),
            )
            return x_t

        def load_tile_staged(t):
            # Warmup path: the first SWDGE loads complete ~50us late (the
            # slow SDMA engines 7/15 fight the SWDGE descriptor-ring writes
            # at startup).  HWDGE is immune, but cannot cast, so stage fp32
            # halves and cast on the (idle during warmup) DVE.
            x_t = xp.tile([128, J * D], bf16)
            src_ap = chunk[t * TILE_ROWS : (t + 1) * TILE_ROWS, :].rearrange(
                "(p j) d -> p (j d)", j=J
            )
            half = J * D // 2
            for h in range(2):
                stage = stagep.tile([128, half], fp32)
                nc.sync.dma_start(out=stage[:], in_=src_ap[:, h * half : (h + 1) * half])
                nc.vector.tensor_copy(x_t[:, h * half : (h + 1) * half], stage[:])
            return x_t

        def scores_front(t, st):
            """DVE: q-product + pair-sum tree + short reduce -> s_t [128,J]."""
            x_t = st["x"]
            prod = pp.tile([128, J * D], bf16)
            nc.vector.tensor_tensor(
                prod[:].rearrange("p (j d) -> p j d", j=J),
                x_t[:].rearrange("p (j d) -> p j d", j=J),
                q_t[:].unsqueeze(1).broadcast_to((128, J, D)),
                Alu.mult,
            )
            h1 = D // 2
            tr1 = t1p.tile([128, J * h1], bf16)
            v1 = tr1[:].rearrange("p (j d) -> p j d", j=J)
            pv = prod[:].rearrange("p (j d) -> p j d", j=J)
            nc.vector.tensor_tensor(v1, pv[:, :, 0:h1], pv[:, :, h1:D], Alu.add)
            s_t = s_p.tile([128, J], fp32)
            nc.vector.tensor_reduce(s_t[:], v1, axis=X, op=Alu.add)
            st["s"] = s_t

        def exp_stage(t, st):
            """ACT exp (unnormalized attn weights, bf16)."""
            e_t = e_p.tile([128, J], fp32)
            nc.scalar.activation(
                out=e_t[:], in_=st["s"], func=Act.Exp, scale=inv_sqrt_d
            )
            st["e"] = e_t

        def softmax_finish(t, st):
            """DVE group sums + reciprocal (normalization happens via the
            per-partition scale on the pooled PSUM->SBUF copy); GpSimd builds
            the UNNORMALIZED diag from exp values."""
            e_t = st["e"]
            gs_t = gs_p.tile([128, G], fp32)
            nc.vector.tensor_reduce(
                gs_t[:], e_t[:].rearrange("p (g r) -> p g r", g=G), axis=X, op=Alu.add
            )
            rec_t = rec_p.tile([128, G], fp32)
            nc.vector.reciprocal(rec_t[:], gs_t[:])
            st["rec"] = rec_t

        DVE_DJ = 8  # diag row-groups built on DVE; the rest on ACT

        def d_stage_dve(t, st):
            d_t = dp.tile([128, J * 128], bf16)
            nc.vector.tensor_tensor(
                d_t[:, : DVE_DJ * 128].rearrange("p (j m) -> p j m", j=DVE_DJ),
                idf_t[:].unsqueeze(1).broadcast_to((128, DVE_DJ, 128)),
                st["e"][:, :DVE_DJ].unsqueeze(2).broadcast_to((128, DVE_DJ, 128)),
                Alu.mult,
            )
            st["d"] = d_t

        def d_stage_act(t, st):
            d_t = st["d"]
            for j in range(DVE_DJ, J):
                nc.scalar.activation(
                    out=d_t[:, j * 128 : (j + 1) * 128],
                    in_=id_t[:],
                    func=Act.Copy,
                    scale=st["e"][:, j : j + 1],
                )

        def pe_block(t, st):
            """PE pool/transpose/proj(+bias) with ACT moves; out store."""
            x_t, d_t, rec_t = st["x"], st["d"], st["rec"]
            out_sb = outp.tile([128, G * D], fp32)
            for g in range(G):
                pool_ps = ps_pool.tile([128, D], fp32)
                for r in range(RATIO):
                    j = g * RATIO + r
                    nc.tensor.matmul(
                        out=pool_ps[:],
                        lhsT=d_t[:, j * 128 : (j + 1) * 128],
                        rhs=x_t[:, j * D : (j + 1) * D],
                        start=(r == 0),
                        stop=(r == RATIO - 1),
                    )
                pooled_sb = pooledp.tile([128, D], bf16)
                nc.scalar.activation(
                    out=pooled_sb[:],
                    in_=pool_ps[:],
                    func=Act.Copy,
                    scale=rec_t[:, g : g + 1],
                )

                pt_ps = ps_pt.tile([128, D], bf16)
                for c in range(4):
                    nc.tensor.transpose(
                        pt_ps[:, c * 128 : (c + 1) * 128],
                        pooled_sb[:, c * 128 : (c + 1) * 128],
                        id_t[:],
                    )
                pt_sb = ptp.tile([128, D], bf16)
                nc.scalar.copy(pt_sb[:], pt_ps[:])

                o_ps = ps_o.tile([128, D], fp32)
                nc.tensor.matmul(
                    out=o_ps[:], lhsT=ones_t[:], rhs=b_t[:], start=True, stop=False
                )
                for c in range(4):
                    nc.tensor.matmul(
                        out=o_ps[:],
                        lhsT=pt_sb[:, c * 128 : (c + 1) * 128],
                        rhs=wt_t[:, c * D : (c + 1) * D],
                        start=False,
                        stop=(c == 3),
                    )
                nc.scalar.copy(out_sb[:, g * D : (g + 1) * D], o_ps[:])
            nc.sync.dma_start(
                out=out[t * 512 : (t + 1) * 512, :].rearrange(
                    "(p j) d -> p (j d)", j=G
                ),
                in_=out_sb[:],
            )

        rep_loop = tc.For_i(0, reps, 1) if reps > 1 else contextlib.nullcontext()
        with rep_loop:
            states = {}
            PREFETCH = 2
            for t in range(min(PREFETCH, n_tiles)):
                states[t] = {"x": load_tile_staged(t)}
            for i in range(n_tiles + 2):
                if i + PREFETCH < n_tiles:
                    states[i + PREFETCH] = {"x": load_tile(i + PREFETCH)}
                if 1 <= i <= n_tiles:
                    # Tile i-1 softmax stages: deps one iteration old, so
                    # exp fires the moment ACT reaches it.
                    exp_stage(i - 1, states[i - 1])
                    softmax_finish(i - 1, states[i - 1])
                    d_stage_dve(i - 1, states[i - 1])
                    d_stage_act(i - 1, states[i - 1])
                if i < n_tiles:
                    scores_front(i, states[i])
                if i >= 2:
                    # PE/copy block lags TWO tiles so its ACT copies never
                    # block the next exp at the ACT queue head.
                    pe_block(i - 2, states[i - 2])
                    del states[i - 2]

    nc.compile()
    return nc


def get_nc(rows_per_core=ROWS_PER_CORE, reps=1):
    key = (rows_per_core, reps)
    if key not in _NC_CACHE:
        _NC_CACHE[key] = _build_nc(rows_per_core, reps)
    return _NC_CACHE[key]


def _aux_inputs(query, w, b):
    import ml_dtypes

    bf16 = ml_dtypes.bfloat16
    q = np.asarray(query, dtype=np.float32)
    qbc = np.ascontiguousarray(np.broadcast_to(q.astype(bf16), (128, D)))
    wtb = np.ascontiguousarray(np.asarray(w, dtype=np.float32).T.astype(bf16))
    ident = np.eye(128, dtype=bf16)
    identf = np.eye(128, dtype=np.float32)
    ones1 = np.ones((1, 128), dtype=bf16)
    brow = np.asarray(b, dtype=np.float32).astype(bf16).reshape(1, D)
    return {
        "qbc": qbc,
        "wtb": wtb,
        "ident": ident,
        "identf": identf,
        "ones1": ones1,
        "brow": brow,
    }


def make_in_maps(chunk, query, w, b, rows_per_core=ROWS_PER_CORE, n_cores=N_CORES):
    chunk = np.asarray(chunk, dtype=np.float32)
    aux = _aux_inputs(query, w, b)
    return [
        {
            "chunk": np.ascontiguousarray(
                chunk[c * rows_per_core : (c + 1) * rows_per_core]
            ),
            **aux,
        }
        for c in range(n_cores)
    ]


def kernel(chunk, query, w, b, trace=False):
    from concourse.bass_utils import run_bass_kernel_spmd

    nc = get_nc(ROWS_PER_CORE)
    in_maps = make_in_maps(chunk, query, w, b)
    res = run_bass_kernel_spmd(nc, in_maps, list(range(N_CORES)), trace=trace)
    out = np.concatenate([res.results[c]["out"] for c in range(N_CORES)], axis=0)
    kernel.last_results = res
    return out
